# revision 2
# baseline (speedup 1.0000x reference)
"""Causal Conv1d (B=8, C=256, T=4096, H=512, K=4) on 8 TRN2 NeuronCores.

Data-parallel over batch: core i computes batch i. Per core:
out[h, t] = sum_{k, c} W[h, c*K+k] * xpad[c, t+k] + bias[h]
as 8 accumulating PE matmuls (contraction = 128 channels, one per
(c_chunk in 2) x (tap k in 4)) per [128h x 512t] fp32 PSUM tile.

Perf design (vs an 82us fp32r first version):
- bf16 operands (host-rounded, ~2.9e-3 rel err vs fp32 reference; output
  returned bf16, upcast on host). fp32r self-loading matmuls serialize a
  107ns weight load with each 213ns stream (measured 0.625 ns/col); bf16
  allows standalone LDWEIGHTS and Fast Weight Load.
- Weight reuse: loops ordered (hj, t-half, q, t-tile) so each 128x128
  weight set is consumed by `tph` consecutive matmuls into different PSUM
  banks. Bass emits one InstLdweights per matmul regardless; the
  _dedup_ldweights pass below deletes the redundant reloads pre-compile
  (verified bit-identical output on HW; nodedup measures ~162us vs ~64us
  because per-matmul separate LDWEIGHTS do not pipeline).
- x (2.1MB) and W (1MB) SBUF-resident, double-buffered across reps of the
  timing loop; input DMAs issue on SP's HW-DGE ring first-needed-first,
  output stores on ACT's ring so prefetch is not FIFO-blocked by stores.
- Bias fused into the PSUM->SBUF evacuation, alternating DVE/ACT.

PE streaming floor is 131072 cols x 0.417ns = 54.6us/core; measured ~60us.
"""

import numpy as np
import ml_dtypes

import concourse.bass as bass
import concourse.mybir as mybir
import concourse.tile as tile
from concourse import bacc
from concourse import bass2jax

B, C, T = 8, 256, 4096
H, K = 512, 4
PAD = K - 1

N_CORES = 8
TT = 512                # t-tile (free dim per matmul, one fp32 PSUM bank)
N_TTILES = T // TT      # 8
N_HCHUNK = H // 128     # 4
N_CCHUNK = C // 128     # 2
N_MM = N_CCHUNK * K     # 8 accumulating matmuls per output tile
TPH = 4                 # t-tiles sharing one weight load (PSUM ping-pong 4+4)
CH = N_MM * 128         # weight columns per h-chunk

_COMPILED = {}


def _dedup_ldweights(nc):
    """Delete InstLdweights that reload the exact weights already in the PE
    array (same weights AP as the previous InstLdweights in the block, no
    semaphore waits/updates attached). Matmuls do not clobber array weights,
    so consecutive same-weight matmuls only need the first load. Must run
    BEFORE nc.compile() (whose move_matmul_waits_to_ldweights pass would
    attach waits to these)."""
    removed = 0
    for f in nc.m.functions:
        for blk in f.blocks:
            insts = blk.instructions
            last_sig = None
            kill = []
            for idx in range(len(insts)):
                inst = insts[idx]
                tn = type(inst).__name__
                if tn == "InstLdweights":
                    sig = (
                        str(inst.ins[0]),
                        str(inst.perf_mode),
                        str(inst.is_transpose),
                        str(inst.tile_position),
                    )
                    si = inst.sync_info
                    clean = si is None or (
                        len(si.on_wait) == 0 and len(si.on_update) == 0
                    )
                    if sig == last_sig and clean:
                        kill.append(idx)
                    else:
                        last_sig = sig
            for idx in reversed(kill):
                del insts[idx]
            removed += len(kill)
    return removed


def _build(reps=1, bias_engine="both", dedup_ldw=True, tph=TPH):
    f32 = mybir.dt.float32
    bf16 = mybir.dt.bfloat16
    nc = bacc.Bacc("TRN2", target_bir_lowering=False, debug=False)

    x_ext = nc.declare_dram_parameter(
        "x", [N_CCHUNK, 128, T + PAD], bf16, isOutput=False
    )
    # wt[hj][c, q*128+m]: lhsT for (q=k*N_CCHUNK+cc, h-chunk hj)
    wt_ext = nc.declare_dram_parameter(
        "wt", [N_HCHUNK, 128, CH], bf16, isOutput=False
    )
    # bias_mat[p, j] = b[j*128 + p]
    b_ext = nc.declare_dram_parameter("bias", [128, N_HCHUNK], f32, isOutput=False)
    out_ext = nc.declare_dram_parameter("out", [H, T], bf16, isOutput=True)

    with tile.TileContext(nc) as tc:
        with (
            tc.tile_pool(name="wpool", bufs=2) as wpool,
            tc.tile_pool(name="xpool", bufs=2) as xpool,
            tc.tile_pool(name="opool", bufs=4) as opool,
            tc.tile_pool(name="psum", bufs=8, space="PSUM") as psum_pool,
        ):

            def store_tile(ps, bt, hj, ti, cnt):
                ot = opool.tile([128, TT], bf16, name="ot", tag="ot")
                if bias_engine == "vector" or (bias_engine == "both" and cnt % 2 == 0):
                    nc.vector.tensor_scalar_add(ot[:], ps[:], bt[:, hj : hj + 1])
                else:
                    nc.scalar.add(ot[:], ps[:], bt[:, hj : hj + 1])
                # stores on ACT's HW-DGE ring; loads own SP's ring
                nc.scalar.dma_start(
                    out_ext[hj * 128 : (hj + 1) * 128, ti * TT : (ti + 1) * TT],
                    ot[:],
                )

            def body():
                # Loads issue on SP's HW-DGE ring, first-needed-first.
                wts = wpool.tile([128, N_HCHUNK * CH], bf16, name="wt", tag="wt")
                bt = wpool.tile([128, N_HCHUNK], f32, name="bias", tag="bias")
                xts = [
                    xpool.tile([128, T + PAD], bf16, name=f"x{cc}", tag=f"x{cc}")
                    for cc in range(N_CCHUNK)
                ]
                HALF = T // 2 + PAD  # 2051
                for cc in range(N_CCHUNK):
                    nc.sync.dma_start(xts[cc][:, 0:HALF], x_ext[cc][:, 0:HALF])
                nc.sync.dma_start(wts[:, 0:CH], wt_ext[0])
                nc.sync.dma_start(bt[:], b_ext[:])
                for cc in range(N_CCHUNK):
                    nc.sync.dma_start(
                        xts[cc][:, HALF : T + PAD], x_ext[cc][:, HALF : T + PAD]
                    )
                for hj in range(1, N_HCHUNK):
                    nc.sync.dma_start(wts[:, hj * CH : (hj + 1) * CH], wt_ext[hj])

                cnt = 0
                for hj in range(N_HCHUNK):
                    for th in range(N_TTILES // tph):
                        pss = [
                            psum_pool.tile([128, TT], f32, name="ps", tag="ps")
                            for _ in range(tph)
                        ]
                        for q in range(N_MM):
                            k, cc = divmod(q, N_CCHUNK)
                            woff = hj * CH + q * 128
                            w_ap = wts[:, woff : woff + 128]
                            for tib in range(tph):
                                t0 = (th * tph + tib) * TT + k
                                nc.tensor.matmul(
                                    pss[tib][:],
                                    w_ap,
                                    xts[cc][:, t0 : t0 + TT],
                                    start=(q == 0),
                                    stop=(q == N_MM - 1),
                                    skip_group_check=True,
                                )
                        for tib in range(tph):
                            store_tile(pss[tib], bt, hj, th * tph + tib, cnt)
                            cnt += 1

            if reps == 1:
                body()
            else:
                with tc.For_i(0, reps, 1):
                    body()

    if dedup_ldw:
        _dedup_ldweights(nc)
    nc.compile()
    return nc


def get_nc():
    if "nc" not in _COMPILED:
        _COMPILED["nc"] = _build()
    return _COMPILED["nc"]


def _prep_inputs(x, W, b):
    x = np.asarray(x, dtype=np.float32)
    W = np.asarray(W, dtype=np.float32)
    b = np.asarray(b, dtype=np.float32)

    xpad = np.zeros((B, N_CCHUNK, 128, T + PAD), dtype=ml_dtypes.bfloat16)
    xpad[:, :, :, PAD:] = x.reshape(B, N_CCHUNK, 128, T).astype(ml_dtypes.bfloat16)

    kern = W.reshape(H, C, K)
    wt = np.empty((N_HCHUNK, 128, CH), dtype=ml_dtypes.bfloat16)
    for hj in range(N_HCHUNK):
        for k in range(K):
            for cc in range(N_CCHUNK):
                q = k * N_CCHUNK + cc
                wt[hj, :, q * 128 : (q + 1) * 128] = (
                    kern[hj * 128 : (hj + 1) * 128, cc * 128 : (cc + 1) * 128, k]
                    .T.astype(ml_dtypes.bfloat16)
                )

    bias_mat = np.ascontiguousarray(b.reshape(N_HCHUNK, 128).T)
    return xpad, wt, bias_mat


def _get_exec():
    if "exec" in _COMPILED:
        return _COMPILED["exec"]

    import jax
    from jax.experimental.shard_map import shard_map
    from jax.sharding import Mesh, PartitionSpec

    nc = get_nc()
    bass2jax.install_neuronx_cc_hook()
    assert nc.dbg_addr is None
    partition_name = nc.partition_id_tensor.name if nc.partition_id_tensor else None

    in_names, out_names, out_avals, zero_outs = [], [], [], []
    for alloc in nc.m.functions[0].allocations:
        if not isinstance(alloc, mybir.MemoryLocationSet):
            continue
        name = alloc.memorylocations[0].name
        if alloc.kind == "ExternalInput":
            if name != partition_name:
                in_names.append(name)
        elif alloc.kind == "ExternalOutput":
            shape = tuple(alloc.tensor_shape)
            dtype = mybir.dt.np(alloc.dtype)
            out_names.append(name)
            out_avals.append(jax.core.ShapedArray(shape, dtype))
            zero_outs.append(np.zeros(shape, dtype))
    n_params = len(in_names)
    all_names = in_names + out_names
    if partition_name is not None:
        all_names = all_names + [partition_name]

    def _body(*args):
        operands = list(args)
        if partition_name is not None:
            operands.append(bass2jax.partition_id_tensor())
        outs = bass2jax._bass_exec_p.bind(
            *operands,
            out_avals=tuple(out_avals),
            in_names=tuple(all_names),
            out_names=tuple(out_names),
            lowering_input_output_aliases=(),
            sim_require_finite=True,
            sim_require_nnan=True,
            nc=nc,
        )
        return tuple(outs)

    devices = jax.devices()[:N_CORES]
    mesh = Mesh(np.asarray(devices), ("core",))
    n_args = n_params + len(out_names)
    sharded = jax.jit(
        shard_map(
            _body,
            mesh=mesh,
            in_specs=(PartitionSpec("core"),) * n_args,
            out_specs=(PartitionSpec("core"),) * len(out_names),
            check_rep=False,
        ),
        keep_unused=True,
    )
    _COMPILED["exec"] = (sharded, in_names, out_names, out_avals, zero_outs, mesh)
    return _COMPILED["exec"]


def _make_args(in_maps):
    sharded, in_names, out_names, out_avals, zero_outs, mesh = _get_exec()
    concat_in = [
        np.concatenate([np.asarray(in_maps[c][nm]) for c in range(N_CORES)], axis=0)
        for nm in in_names
    ]
    concat_zeros = [
        np.zeros((N_CORES * z.shape[0], *z.shape[1:]), z.dtype) for z in zero_outs
    ]
    return concat_in + concat_zeros


def _run(in_maps):
    sharded, in_names, out_names, out_avals, zero_outs, mesh = _get_exec()
    out_arrs = sharded(*_make_args(in_maps))
    return [
        {
            nm: np.asarray(out_arrs[i]).reshape(N_CORES, *out_avals[i].shape)[c]
            for i, nm in enumerate(out_names)
        }
        for c in range(N_CORES)
    ]


def make_in_maps(x, W, b):
    xpad, wt, bias_mat = _prep_inputs(x, W, b)
    return [
        {"x": np.ascontiguousarray(xpad[i]), "wt": wt, "bias": bias_mat}
        for i in range(N_CORES)
    ]


def kernel(x, W, b):
    results = _run(make_in_maps(x, W, b))
    return np.stack(
        [results[i]["out"].astype(np.float32) for i in range(N_CORES)], axis=0
    )
